# revision 27
# baseline (speedup 1.0000x reference)
"""BAM-style attention block (avgpool8 -> 1024-token attention -> nearest-upsample + residual)
as a distributed Bass kernel on 8 TRN2 NeuronCores.

Sharding: core = b*2 + half  (b = batch 0..3, half = H-half 0..1).

Structure (v4):
  phase 1: streams x [512, 128, 256] per row-block on the two HWDGE rings,
           avg-pools 8x8 on DVE (pool scale folded into the weights: 1/64 is
           exact in FP, so the exchanged features are raw bf16 sums), and
           pipelines the pairwise pooled-feature exchange on the gpsimd ring
           (cg0-2 whole groups; cg3 in token halves, doorbells at high
           scheduler priority -- collective completion latency is ~15-20us,
           the single dominant tail term). 20 row-blocks, spread evenly over
           the back half of the stream, are also cast f32->bf16 on DVE into
           persistent SBUF tiles and never re-read in phase 3.
  phase 2: q/k/v projections + 512x1024 attention (bf16). v tiles 6/7
           (remote token-B) accumulate via SBUF adds instead of holding PSUM
           banks, so the full nt0-5 y-train runs before the last exchange
           lands; only ~8us of work trails it. Softmax normalization:
           reciprocal_approx_fast on the [1,512] row-sums + PE broadcast.
           Five phase-3 loads are emitted at high priority before the tail
           so the rings keep streaming through it.
  phase 3: re-streams only the non-resident blocks, adds the upsampled
           attention output on DVE (y kept in bf16; resident adds run
           in-place in the resident tile), writes out in bf16 with stores
           spread over all three rings.
"""

import os
import numpy as np

B, C, H, W = 4, 512, 256, 256
DS = 8
HL = H // 2            # 128 rows per core
IL = HL // DS          # 16 pooled rows per core
WP = W // DS           # 32 pooled cols
NLOC = IL * WP         # 512 local tokens
NH = NLOC // 2         # 256 tokens per exchange half
N = 2 * NLOC           # 1024 tokens
K = C // 8             # 64
CG = C // 128          # 4 channel groups
NT = N // 128          # 8 token tiles (0..3 local, 4..7 remote)
NCI = CG * IL          # 64 row-blocks per core
# Resident row-blocks: kept in SBUF as bf16 through phase 2, never re-read.
# Spread evenly so the extra DVE work (cast + pool) amortizes against the
# DMA-bound stream instead of crawling its tail.
RES_SET = frozenset(range(24, 64, 2))   # 20 blocks
R_RES = len(RES_SET)
STREAM_BUFS = 5        # rotation depth of the shared f32 stream pool
N_EARLY = 5            # phase-3 loads emitted before the attention tail
                       # (<= STREAM_BUFS: deeper would deadlock the issuing
                       # engine behind adds that need y)

_CACHE = {}
TRACE = bool(int(os.environ.get("BAM_TRACE", "0")))
LAST_EXEC_NS = None
LAST_RESULT = None


def _build():
    import concourse.bass as bass
    import concourse.tile as tile
    from concourse import bacc, mybir
    from concourse.masks import make_identity

    f32 = mybir.dt.float32
    bf16 = mybir.dt.bfloat16
    ADD = mybir.AluOpType.add
    SUB = mybir.AluOpType.subtract
    MUL = mybir.AluOpType.mult
    AXY = mybir.AxisListType.XY
    Exp = mybir.ActivationFunctionType.Exp
    POOL_SCALE = 1.0 / (DS * DS)
    GROUPS = [[0, 1], [2, 3], [4, 5], [6, 7]]
    RES_LIST = sorted(RES_SET)

    nc = bacc.Bacc("TRN2", target_bir_lowering=False, debug=False, num_devices=8)

    x_ext = nc.dram_tensor("x", [C, HL, W], f32, kind="ExternalInput")
    wq_ext = nc.dram_tensor("wq", [K, C], f32, kind="ExternalInput")
    bq_ext = nc.dram_tensor("bq", [1, K], f32, kind="ExternalInput")
    wk_ext = nc.dram_tensor("wk", [K, C], f32, kind="ExternalInput")
    bk_ext = nc.dram_tensor("bk", [1, K], f32, kind="ExternalInput")
    wv_ext = nc.dram_tensor("wv", [C, C], f32, kind="ExternalInput")
    bv_ext = nc.dram_tensor("bv", [1, C], f32, kind="ExternalInput")
    out_ext = nc.dram_tensor("out", [C, HL, W], bf16, kind="ExternalOutput")

    with tile.TileContext(nc) as tc:
        with tc.tile_pool(name="persist", bufs=1) as persist, \
             tc.tile_pool(name="scratch", bufs=2) as scratch, \
             tc.tile_pool(name="stream", bufs=STREAM_BUFS) as stream, \
             tc.tile_pool(name="ostage", bufs=3) as ostage, \
             tc.tile_pool(name="psA", bufs=2, space="PSUM") as psA, \
             tc.tile_pool(name="psY", bufs=1, space="PSUM") as psY, \
             tc.tile_pool(name="dram", bufs=1, space="DRAM") as dram:

            # ---- constants ----
            ident = persist.tile([128, 128], bf16, tag="ident")
            make_identity(nc, ident[:])
            ones = persist.tile([1, NLOC], bf16, tag="ones")
            nc.vector.memset(ones[:], 1.0)
            ones_col = persist.tile([128, 1], bf16, tag="ones_col")
            nc.vector.memset(ones_col[:], 1.0)

            # ---- weight DMAs: issue all up-front on the gpsimd ring ----
            b_stage = {}
            for ext, n in ((bq_ext, K), (bk_ext, K), (bv_ext, C)):
                st = scratch.tile([1, n], f32, tag=f"bstage_{ext.name}",
                                  name=f"bstage_{ext.name}", bufs=1)
                nc.gpsimd.dma_start(out=st[:], in_=ext.ap())
                b_stage[ext.name] = st
            w_stage = {}
            for ext in (wq_ext, wk_ext):
                st = scratch.tile([K, C], f32, tag=f"wstage_{ext.name}",
                                  name=f"wstage_{ext.name}", bufs=1)
                nc.gpsimd.dma_start(out=st[:], in_=ext.ap())
                w_stage[ext.name] = st
            wv_stage = []
            for dt in range(CG):
                st = scratch.tile([128, C], f32, tag=f"wvst{dt}", name=f"wvst{dt}",
                                  bufs=1)
                nc.gpsimd.dma_start(out=st[:], in_=wv_ext.ap()[dt * 128:(dt + 1) * 128, :])
                wv_stage.append(st)

            # ---- persistent SBUF tiles ----
            # pooled features as raw bf16 SUMS (scale folded into weights)
            xfb_loc = [persist.tile([128, NLOC], bf16, tag=f"xfl{cg}", name=f"xfl{cg}")
                       for cg in range(CG)]
            xfb_rem = [persist.tile([128, NLOC], bf16, tag=f"xfr{cg}", name=f"xfr{cg}")
                       for cg in range(CG)]
            xres = {ci: persist.tile([128, DS * W], bf16, tag=f"xres{ci}",
                                     name=f"xres{ci}")
                    for ci in RES_LIST}
            # v tiles; vT[6]/vT[7] double as SBUF accumulators during the
            # stream (bf16 adds of per-cg PSUM partials) so no PSUM bank is
            # held hostage until the final exchange
            vT = [persist.tile([128, C], bf16, tag=f"vT{nt}", name=f"vT{nt}")
                  for nt in range(NT)]

            # exchange staging (DRAM; SBUF collectives unsupported)
            xf_loc_d = dram.tile([CG - 1, 128, NLOC], bf16, tag="xf_loc")
            xf_all_d = dram.tile([CG - 1, 2, 128, NLOC], bf16, tag="xf_all")
            xg3_in = [dram.tile([128, NH], bf16, tag=f"xg3i{h}", name=f"xg3i{h}")
                      for h in range(2)]
            xg3_out = [dram.tile([2, 128, NH], bf16, tag=f"xg3o{h}", name=f"xg3o{h}")
                       for h in range(2)]

            # ---- weight prep helpers (casts on DVE; transposes on PE) ----
            bias_b = {}

            def prep_biases():
                for name, n in (("bq", K), ("bk", K), ("bv", C)):
                    bb = persist.tile([1, n], bf16, tag=f"b_{name}", name=f"b_{name}")
                    nc.vector.tensor_copy(out=bb[:], in_=b_stage[name][:])
                    bias_b[name] = bb

            wT = {}

            def prep_qk(ext):
                st = w_stage[ext.name]
                wb = scratch.tile([K, C], bf16, tag=f"wb_{ext.name}",
                                  name=f"wb_{ext.name}", bufs=1)
                nc.vector.tensor_scalar_mul(wb[:], st[:], POOL_SCALE)
                ts = []
                for cg in range(CG):
                    ps = psA.tile([128, K], bf16, tag="s", name=f"tp_{ext.name}{cg}")
                    nc.tensor.transpose(ps[:], wb[:, cg * 128:(cg + 1) * 128],
                                        ident[0:K, 0:K])
                    t = persist.tile([128, K], bf16, tag=f"wT_{ext.name}{cg}",
                                     name=f"wT_{ext.name}{cg}")
                    nc.vector.tensor_copy(out=t[:], in_=ps[:])
                    ts.append(t)
                wT[ext.name] = ts

            wvT = [persist.tile([128, C], bf16, tag=f"wvT{cg}", name=f"wvT{cg}")
                   for cg in range(CG)]

            def prep_wv(dt):
                wvb = scratch.tile([128, C], bf16, tag="wvstage", name=f"wvb{dt}",
                                   bufs=1)
                nc.vector.tensor_scalar_mul(wvb[:], wv_stage[dt][:], POOL_SCALE)
                for cg in range(CG):
                    ps = psA.tile([128, 128], bf16, tag="s", name=f"tpv{dt}{cg}")
                    nc.tensor.transpose(ps[:], wvb[:, cg * 128:(cg + 1) * 128], ident[:])
                    nc.vector.tensor_copy(out=wvT[cg][:, dt * 128:(dt + 1) * 128],
                                          in_=ps[:])

            # PSUM accumulators: biases first so the last in-stream partial
            # completes each sum.
            # Banks: psA = s(2 rotating) + kr + rs = 4;
            #        psY = y0 (vB2) + y1 (vB3) + y2 (spare) + y3 (v67 partials)
            acc = {}

            def prep_psum_init():
                q_ps = psA.tile([K, NLOC], f32, tag="s", name="q_ps")
                nc.tensor.matmul(q_ps[:], bias_b["bq"][:], ones[:], start=True,
                                 stop=False)
                kl_ps = psA.tile([K, NLOC], f32, tag="s", name="kl_ps")
                nc.tensor.matmul(kl_ps[:], bias_b["bk"][:], ones[:], start=True,
                                 stop=False)
                kr_ps = psA.tile([K, NLOC], f32, tag="kr", name="kr_ps", bufs=1)
                nc.tensor.matmul(kr_ps[:], bias_b["bk"][:], ones[:], start=True,
                                 stop=False)
                vB = {}
                for k, nt in ((0, 2), (1, 3)):
                    vB[nt] = psY.tile([128, C], f32, tag=f"y{k}", name=f"vB{nt}")
                    nc.tensor.matmul(vB[nt][:], ones[:, :128], bias_b["bv"][:],
                                     start=True, stop=False)
                acc.update(q=q_ps, kl=kl_ps, kr=kr_ps, vB=vB)

            def vb_partial(nt, cg, src, stop):
                # local token-B v tiles (nt 2,3): PSUM accumulation in psY
                j = nt % 4
                nc.tensor.matmul(acc["vB"][nt][:], src[:, j * 128:(j + 1) * 128],
                                 wvT[cg][:], start=False, stop=stop)

            def v67_partial(cg, first, in_stream):
                # remote token-B v tiles (nt 6,7): per-cg PSUM partial in a
                # transient bank, folded into the bf16 vT accumulator on DVE --
                # keeps two psY banks free so the y-train can run early
                for nt in (6, 7):
                    j = nt % 4
                    if in_stream:
                        vp = psY.tile([128, C], f32, tag="y3", name=f"vp{nt}{cg}")
                    else:
                        vp = psA.tile([128, C], f32, tag="s", name=f"vp{nt}{cg}")
                    if first:
                        nc.tensor.matmul(vp[:], ones[:, :128], bias_b["bv"][:],
                                         start=True, stop=False)
                        nc.tensor.matmul(vp[:], xfb_rem[cg][:, j * 128:(j + 1) * 128],
                                         wvT[cg][:], start=False, stop=True)
                        nc.vector.tensor_copy(out=vT[nt][:], in_=vp[:])
                    else:
                        nc.tensor.matmul(vp[:], xfb_rem[cg][:, j * 128:(j + 1) * 128],
                                         wvT[cg][:], start=True, stop=True)
                        nc.vector.tensor_tensor(out=vT[nt][:], in0=vT[nt][:],
                                                in1=vp[:], op=ADD)

            def remote_recover(cg):
                # partner half = (h0 + h1) - local, recovered rank-agnostically
                xfg = scratch.tile([128, N], bf16, tag="xfg", name=f"xfg{cg}")
                for hf in range(2):
                    nc.gpsimd.dma_start(out=xfg[:, hf * NLOC:(hf + 1) * NLOC],
                                        in_=xf_all_d[cg, hf])
                hsum = scratch.tile([128, NLOC], bf16, tag="hsum", bufs=1,
                                    name=f"hsum{cg}")
                nc.vector.tensor_tensor(out=hsum[:], in0=xfg[:, :NLOC],
                                        in1=xfg[:, NLOC:], op=ADD)
                nc.vector.tensor_tensor(out=xfb_rem[cg][:], in0=hsum[:],
                                        in1=xfb_loc[cg][:], op=SUB)
                nc.tensor.matmul(acc["kr"][:], wT["wk"][cg][:], xfb_rem[cg][:],
                                 start=False, stop=False)
                v67_partial(cg, first=(cg == 0), in_stream=True)

            # ---- phase 1: stream x, pool, exchange; weight prep interleaved ----
            # Exchange staging rides the sync HWDGE ring, emitted two blocks
            # late so its pool-wait never stalls pending load issues, and the
            # gpsimd queue stays free of waits (v4: readbacks waiting on slow
            # collectives blocked later staging -> cascading trigger delays).
            # Partner recovery is deferred until the collective is certainly
            # complete so the DVE queue never stalls mid-stream either.
            deferred = {}

            def defer(ci, fn):
                deferred.setdefault(ci, []).append(fn)

            def stage_group(cg):
                def fn():
                    nc.sync.dma_start(out=xf_loc_d[cg][:],
                                      in_=xfb_loc[cg][:])
                    with tc.high_priority():
                        nc.gpsimd.collective_compute(
                            "AllGather", mybir.AluOpType.bypass,
                            ins=[xf_loc_d[cg].opt()],
                            outs=[xf_all_d[cg].opt()],
                            replica_groups=GROUPS)
                return fn

            def stage_half3(hf):
                def fn():
                    nc.sync.dma_start(
                        out=xg3_in[hf][:],
                        in_=xfb_loc[CG - 1][:, hf * NH:(hf + 1) * NH])
                    with tc.high_priority():
                        nc.gpsimd.collective_compute(
                            "AllGather", mybir.AluOpType.bypass,
                            ins=[xg3_in[hf].opt()],
                            outs=[xg3_out[hf].opt()],
                            replica_groups=GROUPS)
                return fn

            def stream_block(ci):
                for fn in deferred.pop(ci, []):
                    fn()
                cg, ib = divmod(ci, IL)
                x1 = stream.tile([128, DS, W], f32, tag="xs", name=f"x1_{ci}")
                ring = nc.sync if ci % 2 == 0 else nc.scalar
                ring.dma_start(
                    out=x1[:],
                    in_=x_ext.ap()[cg * 128:(cg + 1) * 128,
                                   ib * DS:(ib + 1) * DS, :])
                if ci in RES_SET:
                    xr = xres[ci]
                    nc.vector.tensor_copy(out=xr[:],
                                          in_=x1[:].rearrange("p h w -> p (h w)"))
                    pool_src = xr[:].rearrange("p (h j z) -> p j h z", h=DS, z=DS)
                else:
                    pool_src = x1[:].rearrange("p h (j z) -> p j h z", z=DS)
                with nc.allow_low_precision(
                        reason="8x8 avg-pool sums in bf16; 2e-2 rel-err budget"):
                    nc.vector.tensor_reduce(
                        out=xfb_loc[cg][:, ib * WP:(ib + 1) * WP],
                        in_=pool_src, axis=AXY, op=ADD)
                if ib == IL - 1:
                    # local q/k partials + local token-B v partials
                    last = cg == CG - 1
                    nc.tensor.matmul(acc["q"][:], wT["wq"][cg][:], xfb_loc[cg][:],
                                     start=False, stop=last)
                    nc.tensor.matmul(acc["kl"][:], wT["wk"][cg][:], xfb_loc[cg][:],
                                     start=False, stop=last)
                    for nt in (2, 3):
                        vb_partial(nt, cg, xfb_loc[cg][:], stop=last)

            # NOTE: all PE transposes (tag "s" grabs) must be emitted BEFORE
            # prep_psum_init: q_ps/kl_ps grab "s" slots and hold them through
            # the whole stream.
            # group exchanges fire 2 blocks after the group completes;
            # cg3's A-half fires mid-group, its B-half right after the loop.
            for cg in range(CG - 1):
                defer(cg * IL + IL + 1, stage_group(cg))
            defer(3 * IL + 9, stage_half3(0))
            # partner recovery: only after the collective is certainly done
            # (trigger + ~30us incl. partner skew)
            defer(44, lambda: remote_recover(0))
            defer(52, lambda: remote_recover(1))
            defer(61, lambda: remote_recover(2))

            for ci in range(6):
                stream_block(ci)
            prep_biases()
            for ci in range(6, 8):
                stream_block(ci)
            prep_qk(wq_ext)
            prep_qk(wk_ext)
            for ci in range(8, 12):
                stream_block(ci)
                prep_wv(ci - 8)
            prep_psum_init()
            for ci in range(12, NCI):
                stream_block(ci)
            stage_half3(1)()

            # ---- early phase-3 loads (scalar ring, high priority: the list
            # scheduler otherwise orders them behind the whole attention tail) ----
            x3_tiles = {}
            with tc.high_priority():
                for ei in range(N_EARLY):
                    x3 = stream.tile([128, DS, W], f32, tag="xs", name=f"x3_{ei}")
                    cg, ib = divmod(ei, IL)
                    nc.scalar.dma_start(
                        out=x3[:],
                        in_=x_ext.ap()[cg * 128:(cg + 1) * 128,
                                       ib * DS:(ib + 1) * DS, :])
                    x3_tiles[ei] = x3

            # ================= attention tail =================
            q_sb = persist.tile([K, NLOC], bf16, tag="q_sb")
            nc.vector.tensor_copy(out=q_sb[:], in_=acc["q"][:])
            k_loc = persist.tile([K, NLOC], bf16, tag="k_loc")
            nc.vector.tensor_copy(out=k_loc[:], in_=acc["kl"][:])
            for nt in (2, 3):
                nc.vector.tensor_copy(out=vT[nt][:], in_=acc["vB"][nt][:])

            k_rem = persist.tile([K, NLOC], bf16, tag="k_rem")

            def recover3h(hf):
                sl = slice(hf * NH, (hf + 1) * NH)
                xfg = scratch.tile([128, NLOC], bf16, tag="xfg3", name=f"xfg3{hf}")
                for p in range(2):
                    nc.gpsimd.dma_start(out=xfg[:, p * NH:(p + 1) * NH],
                                        in_=xg3_out[hf][p])
                hsum = scratch.tile([128, NH], bf16, tag="hsum3", name=f"hsum3{hf}")
                nc.vector.tensor_tensor(out=hsum[:], in0=xfg[:, :NH],
                                        in1=xfg[:, NH:], op=ADD)
                nc.vector.tensor_tensor(out=xfb_rem[CG - 1][:, sl], in0=hsum[:],
                                        in1=xfb_loc[CG - 1][:, sl], op=SUB)
                nc.tensor.matmul(acc["kr"][:, sl], wT["wk"][CG - 1][:],
                                 xfb_rem[CG - 1][:, sl], start=False, stop=True)
                nc.vector.tensor_copy(out=k_rem[:, sl], in_=acc["kr"][:, sl])

            # A-half (tokens 0-255) landed mid-stream
            recover3h(0)

            attnT = [persist.tile([128, NLOC], bf16, tag=f"attnT{nt}",
                                  name=f"attnT{nt}")
                     for nt in range(NT)]

            def et_tile(nt):
                ksb = k_loc if nt < 4 else k_rem
                j = nt % 4
                eT_ps = psA.tile([128, NLOC], f32, tag="s", name=f"eT{nt}")
                nc.tensor.matmul(eT_ps[:], ksb[:, j * 128:(j + 1) * 128], q_sb[:],
                                 start=True, stop=True)
                nc.scalar.activation(out=attnT[nt][:], in_=eT_ps[:],
                                     func=Exp, scale=K ** -0.5)

            # energies first (they gate the y-train), then the token-A v tiles
            et_tile(0)
            et_tile(1)

            def vt_tile_full(nt, bank_tag):
                src = xfb_loc if nt < 4 else xfb_rem
                j = nt % 4
                v_ps = psY.tile([128, C], f32, tag=bank_tag, name=f"v_ps{nt}")
                nc.tensor.matmul(v_ps[:], ones[:, :128], bias_b["bv"][:],
                                 start=True, stop=False)
                for cg in range(CG):
                    nc.tensor.matmul(v_ps[:], src[cg][:, j * 128:(j + 1) * 128],
                                     wvT[cg][:], start=False, stop=(cg == CG - 1))
                nc.vector.tensor_copy(out=vT[nt][:], in_=v_ps[:])

            vt_tile_full(0, "y0")
            vt_tile_full(1, "y1")
            vt_tile_full(4, "y2")
            vt_tile_full(5, "y3")

            rs_ps = psA.tile([1, NLOC], f32, tag="rs", name="rs_ps", bufs=1)
            for nt in (2, 3, 4, 5):
                et_tile(nt)
            for nt in (0, 1, 2, 3, 4, 5):
                nc.tensor.matmul(rs_ps[:], ones_col[:], attnT[nt][:],
                                 start=(nt == 0), stop=False)

            # y-train part 1: all four banks are free before the B-half
            # exchange lands, so nt 0-5 complete early
            y_ps = [psY.tile([128, NLOC], f32, tag=f"y{dt}", name=f"yps{dt}")
                    for dt in range(CG)]
            for nt in range(6):
                for dt in range(CG):
                    nc.tensor.matmul(y_ps[dt][:], vT[nt][:, dt * 128:(dt + 1) * 128],
                                     attnT[nt][:], start=(nt == 0), stop=False)

            # B-half (tokens 256-511): recover, finish v6/v7, energies, y.
            # High priority: otherwise the PE queue runs the whole nt0-5
            # y-train before eT6/7 even though their data landed first
            # (observed ~6us of added critical path).
            recover3h(1)
            with tc.high_priority():
                et_tile(6)
                et_tile(7)
                for nt in (6, 7):
                    nc.tensor.matmul(rs_ps[:], ones_col[:], attnT[nt][:],
                                     start=False, stop=(nt == NT - 1))
            v67_partial(CG - 1, first=False, in_stream=False)
            for nt in (6, 7):
                for dt in range(CG):
                    nc.tensor.matmul(y_ps[dt][:], vT[nt][:, dt * 128:(dt + 1) * 128],
                                     attnT[nt][:], start=False, stop=(nt == NT - 1))

            # softmax denominators: fast reciprocal straight off the PSUM
            # row-sums, PE-broadcast, and multiplied in from PSUM -- every
            # copy here is serial critical-path latency before phase 3
            with tc.high_priority():
                ri_f = persist.tile([1, NLOC], f32, tag="ri_f")
                nc.vector.reciprocal_approx_fast(out=ri_f[:], in_=rs_ps[:])
                ri_b = persist.tile([1, NLOC], bf16, tag="ri_b")
                nc.vector.tensor_copy(out=ri_b[:], in_=ri_f[:])
                rb_ps = psA.tile([128, NLOC], f32, tag="s", name="rb_ps")
                nc.tensor.matmul(rb_ps[:], ones[:, :128], ri_b[:], start=True,
                                 stop=True)
                # hw allows only ONE PSUM operand per DVE op; stage via ACT
                rb_sb = persist.tile([128, NLOC], f32, tag="rb_sb")
                nc.scalar.copy(out=rb_sb[:], in_=rb_ps[:])

            y_b = [persist.tile([128, NLOC], bf16, tag=f"yb{dt}", name=f"yb{dt}")
                   for dt in range(CG)]
            for dt in range(CG):
                nc.vector.tensor_tensor(out=y_b[dt][:], in0=y_ps[dt][:],
                                        in1=rb_sb[:], op=MUL)

            # ---- phase 3: out = x + upsample8(y), bf16 stores on all rings ----
            nonres = [ci for ci in range(NCI) if ci not in RES_SET]
            res = RES_LIST
            # burn the prefetched blocks first (frees their stream slots so
            # loads restart immediately), then interleave resident/streamed
            order = [("n", nonres[i]) for i in range(N_EARLY)]
            for i in range(R_RES):
                order.append(("r", res[i]))
                order.append(("n", nonres[N_EARLY + i]))
            for i in range(N_EARLY + R_RES, len(nonres)):
                order.append(("n", nonres[i]))

            st_rings = [nc.gpsimd, nc.sync, nc.scalar]
            n_seen = 0
            for oi, (kind, ci) in enumerate(order):
                cg, ib = divmod(ci, IL)
                # the sync queue is blocked by the last collective's drain
                # until ~y-ready; route the first blocks around it
                early = oi < 6
                yv = y_b[cg][:, ib * WP:(ib + 1) * WP] \
                    [:, None, :, None].broadcast_to([128, DS, WP, DS])
                if kind == "n":
                    if ci in x3_tiles:
                        x3 = x3_tiles[ci]
                    else:
                        x3 = stream.tile([128, DS, W], f32, tag="xs", name=f"x3_{ci}")
                        ring = nc.scalar if early else \
                            (nc.sync if n_seen % 2 == 0 else nc.scalar)
                        ring.dma_start(
                            out=x3[:],
                            in_=x_ext.ap()[cg * 128:(cg + 1) * 128,
                                           ib * DS:(ib + 1) * DS, :])
                    n_seen += 1
                    x3b = ostage.tile([128, DS, W], bf16, tag="x3b", name=f"x3b_{ci}")
                    nc.vector.tensor_tensor(
                        out=x3b[:].rearrange("p h (j z) -> p h j z", z=DS),
                        in0=x3[:].rearrange("p h (j z) -> p h j z", z=DS),
                        in1=yv, op=ADD)
                    src = x3b[:]
                else:
                    xr = xres[ci]
                    xv = xr[:].rearrange("p (h j z) -> p h j z", h=DS, z=DS)
                    nc.vector.tensor_tensor(out=xv, in0=xv, in1=yv, op=ADD)
                    src = xr[:].rearrange("p (h w) -> p h w", h=DS)
                ring = st_rings[(oi % 2) * 2] if early else st_rings[oi % 3]
                ring.dma_start(
                    out=out_ext.ap()[cg * 128:(cg + 1) * 128,
                                     ib * DS:(ib + 1) * DS, :],
                    in_=src)

    nc.finalize()
    return nc


def _get_nc():
    if "nc" not in _CACHE:
        _CACHE["nc"] = _build()
    return _CACHE["nc"]


def kernel(x, Wq, bq, Wk, bk, Wv, bv):
    global LAST_EXEC_NS, LAST_RESULT
    from concourse.bass_utils import run_bass_kernel_spmd

    x = np.asarray(x, dtype=np.float32)
    Wq = np.asarray(Wq, dtype=np.float32)
    bq = np.asarray(bq, dtype=np.float32).reshape(1, K)
    Wk = np.asarray(Wk, dtype=np.float32)
    bk = np.asarray(bk, dtype=np.float32).reshape(1, K)
    Wv = np.asarray(Wv, dtype=np.float32)
    bv = np.asarray(bv, dtype=np.float32).reshape(1, C)

    nc = _get_nc()
    in_maps = []
    for core in range(8):
        b, half = core // 2, core % 2
        in_maps.append({
            "x": np.ascontiguousarray(x[b, :, half * HL:(half + 1) * HL, :]),
            "wq": Wq, "bq": bq, "wk": Wk, "bk": bk, "wv": Wv, "bv": bv,
        })

    res = run_bass_kernel_spmd(nc, in_maps, core_ids=list(range(8)), trace=TRACE)
    LAST_EXEC_NS = res.exec_time_ns
    LAST_RESULT = res

    out = np.empty((B, C, H, W), dtype=np.float32)
    for core in range(8):
        b, half = core // 2, core % 2
        out[b, :, half * HL:(half + 1) * HL, :] = \
            np.asarray(res.results[core]["out"]).astype(np.float32)
    return out
